# revision 17
# baseline (speedup 1.0000x reference)
"""MixLinear int4-GEMM kernel for 8x TRN2 NeuronCores.

Strategy: tensor-parallel over out_features (each core owns OUT/8 = 512
output channels; q_weight / scale_col / weight_cache are sharded along the
output dim; x is replicated).  Per core, per 128-row activation tile:

  1. One custom fused DVE op (registered at build time via the concourse
     custom-DVE Spec API): streams |x * mask| and max-reduces it, giving
     the outlier-masked abs-max in a single [128, 4096] pass without
     materializing the masked tensor.  x_scale = mx/7, r = 1/x_scale.
  2. Magic-number RNE round on the RAW x (outlier columns are zeroed in
     the WEIGHTS instead): t = x*r + 1.5*2^23, q = t - magic -> bf16.
     Both passes are split DVE / ScalarE to balance the engines.
  3. q transposed to contraction-major with one DMA-xbar transpose on the
     Sync HWDGE queue, then converted to fp8e4 (exact for int4 values) on
     ScalarE.
  4. int4 weights unpacked once on-device into fp8 wT [K, 32, 512] with
     outlier rows zeroed; weight_cache pre-divided by scale_col and kept
     f32r.
  5. 16 DoubleRow fp8 matmuls (256-deep contraction each, 2x ALU rate)
     + 2 f32r outlier matmuls accumulate into one PSUM bank; eviction is
     one DVE scalar_tensor_tensor: y = (psum * x_scale) * scale_col.

The output shard [8192, 512] is DMA'd out; the host concatenates shards.
"""

import numpy as np

B, S, IN, OUT, FP = 4, 2048, 4096, 4096, 256
M = B * S
NCORES = 8
OS = OUT // NCORES  # out-features shard per core
QMAX = 7.0
MAGIC = 12582912.0  # 1.5 * 2**23: adding+subtracting forces RNE to integer
SPLIT1 = 1536  # pass-1 columns handled by DVE; remainder on ScalarE
SPLIT2 = 1536  # pass-2 columns handled by DVE; remainder on ScalarE


def _register_absmax_op():
    """Register a fused masked-absmax DVE op: body=|in0*in1|, accum=max.

    Uses the documented concourse custom-DVE extension API (Spec -> DveOp ->
    dve_ops.OPS); the uop sha is computed with the same lower() the table
    generator uses, so the pin is self-consistent.
    """
    from concourse import dve_ops as DO
    from concourse.dve_spec import AluOp, Spec, Src0, Src1, Zero, lower, maxx
    from concourse.dve_uop import DveOpSpec

    name = "TT_ABSMAX_REDUCE_ANT"
    for op in DO.OPS:
        if op.name == name:
            return op

    def _ref(in0, in1, c0, c1, c2):
        b = np.abs(in0.astype(np.float32) * in1.astype(np.float32))
        b = b.reshape(b.shape[0], -1)
        acc = np.maximum(b.max(axis=-1, keepdims=True), 0.0)
        return b, acc

    m = Src0 * Src1
    spec = Spec(
        body=maxx(m, Zero - m), accum=AluOp.MAX, accum_init=Zero, reference=_ref
    )
    row = max(DO._SUB_OPCODE_FOR_NAME.values()) + 1
    assert row < 0x20
    shas = {}
    for ver in ("v3",):  # TRN2
        uops = lower(spec, ver=ver)
        shas[ver] = DveOpSpec(name=name, opcode=row, uops=uops, rd1_en=True).sha(
            ver
        )
    op = DO.DveOp(name, spec, subdim=False, uops_sha=shas)
    DO._SUB_OPCODE_FOR_NAME[name] = row
    DO.OPS.append(op)
    DO.CUSTOM_DVE_SPECS[name] = spec
    return op


def emit_core_kernel(nc, tc, m, in_dim, os_dim, fp_dim):
    """Emit the per-core tile program. All dims compile-time constants."""
    import os as _os

    import concourse.bass as bass
    import concourse.mybir as mybir
    from concourse.masks import make_identity

    DBG = set(_os.environ.get("KERNEL_DISABLE", "").split(","))

    f32 = mybir.dt.float32
    f32r = mybir.dt.float32r
    bf16 = mybir.dt.bfloat16
    f8 = mybir.dt.float8e4
    i32 = mybir.dt.int32
    i16 = mybir.dt.int16
    Alu = mybir.AluOpType
    Act = mybir.ActivationFunctionType
    PM = mybir.MatmulPerfMode

    absmax_op = _register_absmax_op()

    P = 128
    MT = m // P              # number of 128-row activation tiles
    KT = in_dim // P         # number of 128-deep contraction tiles
    FT = fp_dim // P         # outlier contraction tiles (2)
    OJ = os_dim // P         # out-shard subtiles (4)

    x = nc.dram_tensor("x", [m, in_dim], f32, kind="ExternalInput")
    qw = nc.dram_tensor("qw", [os_dim, in_dim // 2], i32, kind="ExternalInput")
    sc = nc.dram_tensor("sc", [os_dim], f32, kind="ExternalInput")
    wc = nc.dram_tensor("wc", [os_dim, fp_dim], f32, kind="ExternalInput")
    maskrow = nc.dram_tensor("maskrow", [in_dim], bf16, kind="ExternalInput")
    idx = nc.dram_tensor("idx", [P, fp_dim // 16], i16, kind="ExternalInput")
    y = nc.dram_tensor("y", [m, os_dim], f32, kind="ExternalOutput")

    with (
        tc.tile_pool(name="const", bufs=1) as const,
        tc.tile_pool(name="wstage", bufs=1) as wstage,
        tc.tile_pool(name="xp", bufs=3) as xp,
        tc.tile_pool(name="tp", bufs=2) as tp,
        tc.tile_pool(name="qp", bufs=2) as qp,
        tc.tile_pool(name="qtp", bufs=2) as qtp,
        tc.tile_pool(name="q8p", bufs=2) as q8p,
        tc.tile_pool(name="aop", bufs=2) as aop,
        tc.tile_pool(name="aotp", bufs=2) as aotp,
        tc.tile_pool(name="sp", bufs=6) as sp,
        tc.tile_pool(name="yp", bufs=2) as yp,
        tc.tile_pool(name="py", bufs=2, space="PSUM") as py,
        tc.tile_pool(name="ptp", bufs=2, space="PSUM") as ptp,
    ):
        # ---------------- one-time setup ----------------
        from concourse import library_config

        if "gather" not in DBG:
            nc.gpsimd.load_library(library_config.ap_gather)

        identity = const.tile([P, P], f32)
        make_identity(nc, identity[:])

        # outlier mask broadcast to all partitions: maskF[p, k] = 0 iff k in ind
        maskF = const.tile([P, in_dim], bf16)
        nc.sync.dma_start(maskF[:], maskrow[None, :].to_broadcast((P, in_dim)))

        # wrapped gather indices for ap_gather
        idxs = const.tile([P, fp_dim // 16], i16)
        nc.sync.dma_start(idxs[:], idx[:])

        # scale_col: broadcast [P, OS] for dequant; per-partition [P, OJ]
        # for pre-dividing weight_cache
        sc_bcast = const.tile([P, os_dim], f32)
        nc.sync.dma_start(sc_bcast[:], sc[None, :].to_broadcast((P, os_dim)))
        sc_op = const.tile([P, OJ], f32)
        nc.sync.dma_start(sc_op[:], sc.rearrange("(j p) -> p j", p=P))

        # weight_cache': wc[o, f] / sc[o], transposed to [P_f, FT, OS] f32r
        wc_sb = wstage.tile([P, OJ, fp_dim], f32)
        nc.sync.dma_start(wc_sb[:], wc.rearrange("(j p) f -> p j f", p=P))
        rsc_op = const.tile([P, OJ], f32)
        nc.vector.reciprocal(rsc_op[:], sc_op[:])
        wcT = const.tile([P, FT, os_dim], f32r)
        for j in range(OJ):
            wcpj = wstage.tile([P, fp_dim], f32, tag="wcpj")
            nc.vector.tensor_scalar(
                wcpj[:], wc_sb[:, j, :], rsc_op[:, j : j + 1], None, Alu.mult
            )
            for ff in range(FT):
                ps = ptp.tile([P, P], f32, tag="tp")
                nc.tensor.transpose(
                    ps[:], wcpj[:, ff * P : (ff + 1) * P], identity[:]
                )
                nc.scalar.activation(
                    wcT[:, ff, j * P : (j + 1) * P], ps[:], Act.Copy
                )

        # int4 weight unpack: qw[o, i] byte -> w[o, 2i] = lo nibble signed,
        # w[o, 2i+1] = hi nibble signed; outlier k-rows zeroed via mask
        # multiply; DMA-xbar transpose (bf16), then convert to fp8 wT
        # [P_k, KT, OS] (int values, exact in e4m3).
        wT8 = const.tile([P, KT, os_dim], f8)
        qw_v = qw.rearrange("(j p) k -> p j k", p=P)
        for j in range(OJ):
            qwj = wstage.tile([P, in_dim // 2], i32, tag="qwj")
            nc.sync.dma_start(qwj[:], qw_v[:, j, :])
            w_ok = wstage.tile([P, in_dim], bf16, tag="wok")
            w_ok_v = w_ok.rearrange("p (k two) -> p k two", two=2)
            tmp = wstage.tile([P, in_dim // 2], i32, tag="wtmp")
            # high nibble first so `tmp` can be reused for the low nibble
            nc.vector.tensor_scalar(
                tmp[:], qwj[:], 4, None, Alu.arith_shift_right
            )
            nc.vector.tensor_scalar(
                tmp[:], tmp[:], 15, 8, Alu.bitwise_and, Alu.bitwise_xor
            )
            nc.vector.tensor_scalar(w_ok_v[:, :, 1], tmp[:], 8, None, Alu.subtract)
            # low nibble: ((v & 15) ^ 8) - 8
            nc.vector.tensor_scalar(
                tmp[:], qwj[:], 15, 8, Alu.bitwise_and, Alu.bitwise_xor
            )
            nc.vector.tensor_scalar(w_ok_v[:, :, 0], tmp[:], 8, None, Alu.subtract)
            # zero outlier k-columns so raw-x quantization error there is inert
            w_okm = wstage.tile([P, in_dim], bf16, tag="wokm")
            nc.vector.tensor_tensor(w_okm[:], w_ok[:], maskF[:], Alu.mult)
            # transpose [128 o, in_dim k] -> bf16 staging, convert to fp8
            wt16 = wstage.tile([P, KT, P], bf16, tag="wt16")
            nc.sync.dma_start_transpose(wt16[:], w_okm[:])
            nc.vector.tensor_copy(wT8[:, :, j * P : (j + 1) * P], wt16[:])

        # ---------------- main loop over 128-row activation tiles ----------
        for mi in range(MT):
            x_t = xp.tile([P, in_dim], f32)
            nc.sync.dma_start(x_t[:], x[mi * P : (mi + 1) * P, :])

            # fused masked abs-max: mx = max_k |x * mask| in one DVE pass
            mx = sp.tile([P, 1], f32, tag="mx")
            dummy = sp.tile([P, 1], f32, tag="dummy")
            nc.vector._custom_dve(
                absmax_op,
                out=dummy[:].broadcast_to((P, in_dim)),
                in0=x_t[:],
                in1=maskF[:],
                accum_out=mx[:],
            )
            s_t = sp.tile([P, 1], f32, tag="s")
            nc.vector.tensor_scalar(
                s_t[:], mx[:], float(np.float32(1.0) / np.float32(QMAX)), None,
                Alu.mult,
            )
            r_t = sp.tile([P, 1], f32, tag="r")
            nc.vector.reciprocal(r_t[:], s_t[:])

            # outlier activations: gather + pre-scale by r, transpose via PE
            ao = aop.tile([P, fp_dim], f32, tag="ao")
            if "gather" in DBG:
                nc.vector.tensor_copy(ao[:], x_t[:, :fp_dim])
            else:
                nc.gpsimd.ap_gather(
                    ao[:, :, None],
                    x_t[:, :, None],
                    idxs[:],
                    channels=P,
                    num_elems=in_dim,
                    d=1,
                    num_idxs=fp_dim,
                )
            aos = aop.tile([P, fp_dim], f32, tag="aos")
            nc.vector.tensor_scalar(aos[:], ao[:], r_t[:], None, Alu.mult)
            aoT = aotp.tile([P, FT, P], f32r)
            for ff in range(FT):
                ps = ptp.tile([P, P], f32, tag="tp")
                nc.tensor.transpose(
                    ps[:], aos[:, ff * P : (ff + 1) * P], identity[:]
                )
                nc.scalar.activation(aoT[:, ff, :], ps[:], Act.Copy)

            # quantize pass 1: t = x * r + MAGIC (fp32), split DVE / ScalarE
            t_t = tp.tile([P, in_dim], f32)
            nc.vector.tensor_scalar(
                t_t[:, :SPLIT1], x_t[:, :SPLIT1], r_t[:], MAGIC,
                Alu.mult, Alu.add,
            )
            nc.scalar.activation(
                t_t[:, SPLIT1:], x_t[:, SPLIT1:], Act.Copy, bias=MAGIC,
                scale=r_t[:],
            )
            # quantize pass 2: q = t - MAGIC -> bf16, split DVE / ScalarE
            q = qp.tile([P, in_dim], bf16)
            nc.vector.tensor_scalar(
                q[:, :SPLIT2], t_t[:, :SPLIT2], -MAGIC, None, Alu.add
            )
            nc.scalar.activation(
                q[:, SPLIT2:], t_t[:, SPLIT2:], Act.Copy, bias=-MAGIC
            )

            # transpose q to contraction-major via DMA xbar (sync queue —
            # the ~4us descriptor generation must not block ScalarE compute),
            # then convert to fp8 (exact: values are small ints)
            qT = qtp.tile([P, KT, P], bf16)
            nc.sync.dma_start_transpose(qT[:], q[:])
            qT8 = q8p.tile([P, KT, P], f8)
            qT_f = qT.rearrange("p a b -> p (a b)")
            qT8_f = qT8.rearrange("p a b -> p (a b)")
            nc.vector.tensor_scalar(
                qT8_f[:, :1536], qT_f[:, :1536], 1.0, None, Alu.mult
            )
            nc.scalar.activation(qT8_f[:, 1536:], qT_f[:, 1536:], Act.Copy)

            # GEMMs: 16 DoubleRow fp8 (256-deep) + 2 f32r outlier tiles
            psum = py.tile([P, os_dim], f32)
            for ko in range(0, KT, 2):
                nc.tensor.matmul(
                    psum[:],
                    qT8[:, ko : ko + 2, :],
                    wT8[:, ko : ko + 2, :],
                    start=(ko == 0),
                    stop=False,
                    perf_mode=PM.DoubleRow,
                )
            for ff in range(FT):
                nc.tensor.matmul(
                    psum[:],
                    aoT[:, ff, :],
                    wcT[:, ff, :],
                    start=False,
                    stop=(ff == FT - 1),
                )

            # dequant + store in one DVE op: y = (psum * x_scale) * scale_col
            yt = yp.tile([P, os_dim], f32, tag="yt")
            nc.vector.scalar_tensor_tensor(
                yt[:], psum[:], s_t[:], sc_bcast[:], Alu.mult, Alu.mult
            )
            nc.scalar.dma_start(y[mi * P : (mi + 1) * P, :], yt[:])

    return nc


def build_nc(m=M, in_dim=IN, os_dim=OS, fp_dim=FP):
    import concourse.bacc as bacc
    import concourse.tile as tile

    nc = bacc.Bacc(None, target_bir_lowering=False)
    with tile.TileContext(nc) as tc:
        emit_core_kernel(nc, tc, m, in_dim, os_dim, fp_dim)
    nc.compile()
    return nc


def make_host_inputs(x, q_weight, scale_col, weight_cache, ind,
                     m=M, in_dim=IN, os_dim=OS, fp_dim=FP, ncores=NCORES):
    """Shard/relayout full inputs into per-core input maps (no arithmetic)."""
    import ml_dtypes

    xf = np.ascontiguousarray(x.reshape(m, in_dim).astype(np.float32, copy=False))
    ind = np.asarray(ind).astype(np.int64)
    maskrow = np.ones(in_dim, dtype=np.float32)
    maskrow[ind] = 0.0
    maskrow = maskrow.astype(ml_dtypes.bfloat16)
    w = ind.astype(np.int16).reshape(fp_dim // 16, 16)  # j = i*16 + (p%16)
    idx = np.tile(w.T, (8, 1)).astype(np.int16)  # [128, fp/16]
    scf = np.asarray(scale_col).reshape(-1).astype(np.float32, copy=False)

    in_maps = []
    for c in range(ncores):
        o0, o1 = c * os_dim, (c + 1) * os_dim
        in_maps.append(
            {
                "x": xf,
                "qw": np.ascontiguousarray(q_weight[o0:o1]).astype(np.int32, copy=False),
                "sc": np.ascontiguousarray(scf[o0:o1]),
                "wc": np.ascontiguousarray(weight_cache[o0:o1]).astype(np.float32, copy=False),
                "maskrow": maskrow,
                "idx": idx,
            }
        )
    return in_maps


_NC_CACHE = {}


def kernel(x, q_weight, scale_col, weight_cache, ind, trace=False):
    from concourse.bass_utils import run_bass_kernel_spmd

    key = "full"
    if key not in _NC_CACHE:
        _NC_CACHE[key] = build_nc()
    nc = _NC_CACHE[key]

    in_maps = make_host_inputs(x, q_weight, scale_col, weight_cache, ind)
    res = run_bass_kernel_spmd(nc, in_maps, list(range(NCORES)), trace=trace)
    yshards = [res.results[c]["y"] for c in range(NCORES)]
    yfull = np.concatenate(yshards, axis=1).reshape(B, S, OUT)
    if trace:
        return yfull, res
    return yfull


# revision 20
# speedup vs baseline: 1.1762x; 1.1762x over previous
"""MixLinear int4-GEMM kernel for 8x TRN2 NeuronCores.

Strategy: tensor-parallel over out_features (each core owns OUT/8 = 512
output channels; q_weight / scale_col / weight_cache are sharded along the
output dim; x is replicated).  Per core, per 128-row activation tile:

  1. One custom fused DVE op (registered at build time via the concourse
     custom-DVE Spec API): streams |x * mask| and max-reduces it, giving
     the outlier-masked abs-max in a single [128, 4096] pass without
     materializing the masked tensor.  x_scale = mx/7, r = 1/x_scale.
  2. Magic-number RNE round on the RAW x (outlier columns are zeroed in
     the WEIGHTS instead): t = x*r + 1.5*2^23, q = t - magic -> bf16.
     Both passes are split DVE / ScalarE to balance the engines.
  3. q transposed to contraction-major with one DMA-xbar transpose on the
     Sync HWDGE queue, then converted to fp8e4 (exact for int4 values) on
     ScalarE.
  4. int4 weights unpacked once on-device into fp8 wT [K, 32, 512] with
     outlier rows zeroed; weight_cache pre-divided by scale_col and kept
     f32r.
  5. 16 DoubleRow fp8 matmuls (256-deep contraction each, 2x ALU rate)
     + 2 f32r outlier matmuls accumulate into one PSUM bank; eviction is
     one DVE scalar_tensor_tensor: y = (psum * x_scale) * scale_col.

The output shard [8192, 512] is DMA'd out; the host concatenates shards.
"""

import numpy as np

B, S, IN, OUT, FP = 4, 2048, 4096, 4096, 256
M = B * S
NCORES = 8
OS = OUT // NCORES  # out-features shard per core
QMAX = 7.0
MAGIC = 12582912.0  # 1.5 * 2**23: adding+subtracting forces RNE to integer
SPLIT1 = 1536  # pass-1 columns handled by DVE; remainder on ScalarE
SPLIT2 = 1536  # pass-2 columns handled by DVE; remainder on ScalarE


def _register_absmax_op():
    """Register a fused masked-absmax DVE op: body=|in0*in1|, accum=max.

    Uses the documented concourse custom-DVE extension API (Spec -> DveOp ->
    dve_ops.OPS); the uop sha is computed with the same lower() the table
    generator uses, so the pin is self-consistent.
    """
    from concourse import dve_ops as DO
    from concourse.dve_spec import AluOp, Spec, Src0, Src1, Zero, lower, maxx
    from concourse.dve_uop import DveOpSpec

    name = "TT_ABSMAX_REDUCE_ANT"
    for op in DO.OPS:
        if op.name == name:
            return op

    def _ref(in0, in1, c0, c1, c2):
        b = np.abs(in0.astype(np.float32) * in1.astype(np.float32))
        b = b.reshape(b.shape[0], -1)
        acc = np.maximum(b.max(axis=-1, keepdims=True), 0.0)
        return b, acc

    m = Src0 * Src1
    spec = Spec(
        body=maxx(m, Zero - m), accum=AluOp.MAX, accum_init=Zero, reference=_ref
    )
    row = max(DO._SUB_OPCODE_FOR_NAME.values()) + 1
    assert row < 0x20
    shas = {}
    for ver in ("v3",):  # TRN2
        uops = lower(spec, ver=ver)
        shas[ver] = DveOpSpec(name=name, opcode=row, uops=uops, rd1_en=True).sha(
            ver
        )
    op = DO.DveOp(name, spec, subdim=False, uops_sha=shas)
    DO._SUB_OPCODE_FOR_NAME[name] = row
    DO.OPS.append(op)
    DO.CUSTOM_DVE_SPECS[name] = spec
    return op


def emit_core_kernel(nc, tc, m, in_dim, os_dim, fp_dim):
    """Emit the per-core tile program. All dims compile-time constants."""
    import os as _os

    import concourse.bass as bass
    import concourse.mybir as mybir
    from concourse.masks import make_identity

    DBG = set(_os.environ.get("KERNEL_DISABLE", "").split(","))

    f32 = mybir.dt.float32
    f32r = mybir.dt.float32r
    bf16 = mybir.dt.bfloat16
    f8 = mybir.dt.float8e4
    i32 = mybir.dt.int32
    i16 = mybir.dt.int16
    Alu = mybir.AluOpType
    Act = mybir.ActivationFunctionType
    PM = mybir.MatmulPerfMode

    absmax_op = _register_absmax_op()

    P = 128
    MT = m // P              # number of 128-row activation tiles
    KT = in_dim // P         # number of 128-deep contraction tiles
    FT = fp_dim // P         # outlier contraction tiles (2)
    OJ = os_dim // P         # out-shard subtiles (4)

    x = nc.dram_tensor("x", [m, in_dim], f32, kind="ExternalInput")
    qw = nc.dram_tensor("qw", [os_dim, in_dim // 2], i32, kind="ExternalInput")
    sc = nc.dram_tensor("sc", [os_dim], f32, kind="ExternalInput")
    wc = nc.dram_tensor("wc", [os_dim, fp_dim], f32, kind="ExternalInput")
    maskrow = nc.dram_tensor("maskrow", [in_dim], bf16, kind="ExternalInput")
    idx = nc.dram_tensor("idx", [P, fp_dim // 16], i16, kind="ExternalInput")
    y = nc.dram_tensor("y", [m, os_dim], f32, kind="ExternalOutput")

    with (
        tc.tile_pool(name="const", bufs=1) as const,
        tc.tile_pool(name="wstage", bufs=1) as wstage,
        tc.tile_pool(name="xp", bufs=3) as xp,
        tc.tile_pool(name="tp", bufs=2) as tp,
        tc.tile_pool(name="qp", bufs=2) as qp,
        tc.tile_pool(name="qtp", bufs=2) as qtp,
        tc.tile_pool(name="q8p", bufs=3) as q8p,
        tc.tile_pool(name="aop", bufs=2) as aop,
        tc.tile_pool(name="aotp", bufs=2) as aotp,
        tc.tile_pool(name="sp", bufs=6) as sp,
        tc.tile_pool(name="yp", bufs=2) as yp,
        tc.tile_pool(name="py", bufs=3, space="PSUM") as py,
        tc.tile_pool(name="ptp", bufs=2, space="PSUM") as ptp,
    ):
        # ---------------- one-time setup ----------------
        from concourse import library_config

        if "gather" not in DBG:
            nc.gpsimd.load_library(library_config.ap_gather)

        identity = const.tile([P, P], f32)
        make_identity(nc, identity[:])

        # outlier mask broadcast to all partitions: maskF[p, k] = 0 iff k in ind
        maskF = const.tile([P, in_dim], bf16)
        nc.sync.dma_start(maskF[:], maskrow[None, :].to_broadcast((P, in_dim)))

        # wrapped gather indices for ap_gather
        idxs = const.tile([P, fp_dim // 16], i16)
        nc.sync.dma_start(idxs[:], idx[:])

        # scale_col: broadcast [P, OS] for dequant; per-partition [P, OJ]
        # for pre-dividing weight_cache
        sc_bcast = const.tile([P, os_dim], f32)
        nc.sync.dma_start(sc_bcast[:], sc[None, :].to_broadcast((P, os_dim)))
        sc_op = const.tile([P, OJ], f32)
        nc.sync.dma_start(sc_op[:], sc.rearrange("(j p) -> p j", p=P))

        # weight_cache': wc[o, f] / sc[o], transposed to [P_f, FT, OS] f32r
        wc_v = wc.rearrange("(j p) f -> p j f", p=P)
        rsc_op = const.tile([P, OJ], f32)
        nc.vector.reciprocal(rsc_op[:], sc_op[:])
        wcT = const.tile([P, FT, os_dim], f32r)
        for j in range(OJ):
            wc_sbj = wstage.tile([P, fp_dim], f32, tag="wcsb")
            nc.sync.dma_start(wc_sbj[:], wc_v[:, j, :])
            wcpj = wstage.tile([P, fp_dim], f32, tag="wcpj")
            nc.vector.tensor_scalar(
                wcpj[:], wc_sbj[:], rsc_op[:, j : j + 1], None, Alu.mult
            )
            for ff in range(FT):
                ps = ptp.tile([P, P], f32, tag="tp")
                nc.tensor.transpose(
                    ps[:], wcpj[:, ff * P : (ff + 1) * P], identity[:]
                )
                nc.scalar.activation(
                    wcT[:, ff, j * P : (j + 1) * P], ps[:], Act.Copy
                )

        # int4 weight unpack: qw[o, i] byte -> w[o, 2i] = lo nibble signed,
        # w[o, 2i+1] = hi nibble signed; outlier k-rows zeroed via mask
        # multiply; DMA-xbar transpose (bf16), then convert to fp8 wT
        # [P_k, KT, OS] (int values, exact in e4m3).
        wT8 = const.tile([P, KT, os_dim], f8)
        qw_v = qw.rearrange("(j p) k -> p j k", p=P)
        for j in range(OJ):
            qwj = wstage.tile([P, in_dim // 2], i32, tag="qwj")
            nc.sync.dma_start(qwj[:], qw_v[:, j, :])
            w_ok = wstage.tile([P, in_dim], bf16, tag="wok")
            w_ok_v = w_ok.rearrange("p (k two) -> p k two", two=2)
            tmp = wstage.tile([P, in_dim // 2], i32, tag="wtmp")
            # high nibble first so `tmp` can be reused for the low nibble
            nc.vector.tensor_scalar(
                tmp[:], qwj[:], 4, None, Alu.arith_shift_right
            )
            nc.vector.tensor_scalar(
                tmp[:], tmp[:], 15, 8, Alu.bitwise_and, Alu.bitwise_xor
            )
            nc.vector.tensor_scalar(w_ok_v[:, :, 1], tmp[:], 8, None, Alu.subtract)
            # low nibble: ((v & 15) ^ 8) - 8
            nc.vector.tensor_scalar(
                tmp[:], qwj[:], 15, 8, Alu.bitwise_and, Alu.bitwise_xor
            )
            nc.vector.tensor_scalar(w_ok_v[:, :, 0], tmp[:], 8, None, Alu.subtract)
            # zero outlier k-columns so raw-x quantization error there is inert
            w_okm = wstage.tile([P, in_dim], bf16, tag="wokm")
            nc.vector.tensor_tensor(w_okm[:], w_ok[:], maskF[:], Alu.mult)
            # transpose [128 o, in_dim k] -> bf16 staging, convert to fp8
            wt16 = wstage.tile([P, KT, P], bf16, tag="wt16")
            nc.sync.dma_start_transpose(wt16[:], w_okm[:])
            nc.vector.tensor_copy(wT8[:, :, j * P : (j + 1) * P], wt16[:])

        # ---------------- main loop over 128-row activation tiles ----------
        # stats (absmax -> s -> r) are emitted ONE TILE AHEAD of the bulk
        # work so ScalarE never parks waiting for r_t: by the time tile i's
        # quantize reaches the Scalar queue head, r(i) was produced during
        # tile i-1's bulk phase.
        carry = {}

        def load_stats(i):
            x_t = xp.tile([P, in_dim], f32)
            nc.sync.dma_start(x_t[:], x[i * P : (i + 1) * P, :])
            mx = sp.tile([P, 1], f32, tag="mx")
            dummy = sp.tile([P, 1], f32, tag="dummy")
            nc.vector._custom_dve(
                absmax_op,
                out=dummy[:].broadcast_to((P, in_dim)),
                in0=x_t[:],
                in1=maskF[:],
                accum_out=mx[:],
            )
            s_t = sp.tile([P, 1], f32, tag="s")
            nc.vector.tensor_scalar(
                s_t[:], mx[:], float(np.float32(1.0) / np.float32(QMAX)), None,
                Alu.mult,
            )
            r_t = sp.tile([P, 1], f32, tag="r")
            nc.vector.reciprocal(r_t[:], s_t[:])
            carry[i] = (x_t, s_t, r_t)

        load_stats(0)
        for mi in range(MT):
            if mi + 1 < MT:
                load_stats(mi + 1)
            x_t, s_t, r_t = carry.pop(mi)

            # outlier activations: gather + pre-scale by r, transpose via PE
            ao = aop.tile([P, fp_dim], f32, tag="ao")
            if "gather" in DBG:
                nc.vector.tensor_copy(ao[:], x_t[:, :fp_dim])
            else:
                nc.gpsimd.ap_gather(
                    ao[:, :, None],
                    x_t[:, :, None],
                    idxs[:],
                    channels=P,
                    num_elems=in_dim,
                    d=1,
                    num_idxs=fp_dim,
                )
            aos = aop.tile([P, fp_dim], f32, tag="aos")
            nc.vector.tensor_scalar(aos[:], ao[:], r_t[:], None, Alu.mult)
            aoT = aotp.tile([P, FT, P], f32r)
            for ff in range(FT):
                ps = ptp.tile([P, P], f32, tag="tp")
                nc.tensor.transpose(
                    ps[:], aos[:, ff * P : (ff + 1) * P], identity[:]
                )
                nc.scalar.activation(aoT[:, ff, :], ps[:], Act.Copy)

            # quantize pass 1: t = x * r + MAGIC (fp32), split DVE / ScalarE
            t_t = tp.tile([P, in_dim], f32)
            nc.vector.tensor_scalar(
                t_t[:, :SPLIT1], x_t[:, :SPLIT1], r_t[:], MAGIC,
                Alu.mult, Alu.add,
            )
            nc.scalar.activation(
                t_t[:, SPLIT1:], x_t[:, SPLIT1:], Act.Copy, bias=MAGIC,
                scale=r_t[:],
            )
            # quantize pass 2: q = t - MAGIC -> bf16, split DVE / ScalarE
            q = qp.tile([P, in_dim], bf16)
            nc.vector.tensor_scalar(
                q[:, :SPLIT2], t_t[:, :SPLIT2], -MAGIC, None, Alu.add
            )
            nc.scalar.activation(
                q[:, SPLIT2:], t_t[:, SPLIT2:], Act.Copy, bias=-MAGIC
            )

            # transpose q to contraction-major via DMA xbar (sync queue),
            # then convert to fp8 (exact: values are small ints)
            qT = qtp.tile([P, KT, P], bf16)
            nc.sync.dma_start_transpose(qT[:], q[:])
            qT8 = q8p.tile([P, KT, P], f8)
            nc.scalar.activation(qT8[:], qT[:], Act.Copy)

            # GEMMs: 16 DoubleRow fp8 (256-deep) + 2 f32r outlier tiles
            psum = py.tile([P, os_dim], f32)
            for ko in range(0, KT, 2):
                nc.tensor.matmul(
                    psum[:],
                    qT8[:, ko : ko + 2, :],
                    wT8[:, ko : ko + 2, :],
                    start=(ko == 0),
                    stop=False,
                    perf_mode=PM.DoubleRow,
                )
            for ff in range(FT):
                nc.tensor.matmul(
                    psum[:],
                    aoT[:, ff, :],
                    wcT[:, ff, :],
                    start=False,
                    stop=(ff == FT - 1),
                )

            # dequant + store in one DVE op: y = (psum * x_scale) * scale_col
            yt = yp.tile([P, os_dim], f32, tag="yt")
            nc.vector.scalar_tensor_tensor(
                yt[:], psum[:], s_t[:], sc_bcast[:], Alu.mult, Alu.mult
            )
            nc.scalar.dma_start(y[mi * P : (mi + 1) * P, :], yt[:])

    return nc


def build_nc(m=M, in_dim=IN, os_dim=OS, fp_dim=FP):
    import concourse.bacc as bacc
    import concourse.tile as tile

    nc = bacc.Bacc(None, target_bir_lowering=False)
    with tile.TileContext(nc) as tc:
        emit_core_kernel(nc, tc, m, in_dim, os_dim, fp_dim)
    nc.compile()
    return nc


def make_host_inputs(x, q_weight, scale_col, weight_cache, ind,
                     m=M, in_dim=IN, os_dim=OS, fp_dim=FP, ncores=NCORES):
    """Shard/relayout full inputs into per-core input maps (no arithmetic)."""
    import ml_dtypes

    xf = np.ascontiguousarray(x.reshape(m, in_dim).astype(np.float32, copy=False))
    ind = np.asarray(ind).astype(np.int64)
    maskrow = np.ones(in_dim, dtype=np.float32)
    maskrow[ind] = 0.0
    maskrow = maskrow.astype(ml_dtypes.bfloat16)
    w = ind.astype(np.int16).reshape(fp_dim // 16, 16)  # j = i*16 + (p%16)
    idx = np.tile(w.T, (8, 1)).astype(np.int16)  # [128, fp/16]
    scf = np.asarray(scale_col).reshape(-1).astype(np.float32, copy=False)

    in_maps = []
    for c in range(ncores):
        o0, o1 = c * os_dim, (c + 1) * os_dim
        in_maps.append(
            {
                "x": xf,
                "qw": np.ascontiguousarray(q_weight[o0:o1]).astype(np.int32, copy=False),
                "sc": np.ascontiguousarray(scf[o0:o1]),
                "wc": np.ascontiguousarray(weight_cache[o0:o1]).astype(np.float32, copy=False),
                "maskrow": maskrow,
                "idx": idx,
            }
        )
    return in_maps


_NC_CACHE = {}


def kernel(x, q_weight, scale_col, weight_cache, ind, trace=False):
    from concourse.bass_utils import run_bass_kernel_spmd

    key = "full"
    if key not in _NC_CACHE:
        _NC_CACHE[key] = build_nc()
    nc = _NC_CACHE[key]

    in_maps = make_host_inputs(x, q_weight, scale_col, weight_cache, ind)
    res = run_bass_kernel_spmd(nc, in_maps, list(range(NCORES)), trace=trace)
    yshards = [res.results[c]["y"] for c in range(NCORES)]
    yfull = np.concatenate(yshards, axis=1).reshape(B, S, OUT)
    if trace:
        return yfull, res
    return yfull
